# revision 1
# baseline (speedup 1.0000x reference)
"""4-layer tanh RNN on 8 Trainium2 NeuronCores.

Strategy: 4-stage layer pipeline x 2-way batch split. Core c handles
layer c//2 for batch half c%2. Time is processed in blocks of T=32 steps;
each round every core: gathers its input block (previous stage's output)
from the round's AllGather, projects it (x @ WxT + b), runs 32 recurrence
steps (weight-stationary bf16 matmuls, zT[d_out,b] layout so h never needs
a transpose), then contributes its output block to the next AllGather.
Cross-core addressing is SPMD-uniform: per-core *data* (indirect-DMA gather
indices, carry/init masks, zeroed feeds) encodes each core's role.

Compute dtype bf16 (PE fp32 is 4x slower), fp32 PSUM accumulation, fp32
tanh. Measured end-to-end absmax-relative error vs the fp32 reference
~1e-2.
"""
import sys
import numpy as np

if "/opt/trn_rl_repo" not in sys.path:
    sys.path.insert(0, "/opt/trn_rl_repo")

import ml_dtypes

BF = ml_dtypes.bfloat16

# Problem config (hardcoded per contract)
import os as _os
B, L, D, NL = 16, 512, 1024, 4
if _os.environ.get("RNN_SMALL"):  # dev-only fast config; L shrinks
    L = int(_os.environ["RNN_SMALL"])
P = 128
KT = D // P          # 8 k-tiles (contraction)
MT = D // P          # 8 m-tiles (output)
BC = B // 2          # 8 = per-core batch half
T = 32               # timesteps per block
NB = L // T          # 16 blocks
ROUNDS = NB + NL - 1  # 19
N_CORES = 8
BLK_COLS = MT * T * BC  # 2048 block columns: col = m*T*BC + t*BC + b

_cache = {}


def _build():
    import concourse.bass as bass
    import concourse.mybir as mybir
    import concourse.tile as tile
    from concourse import bacc
    from concourse.tile import add_dep_helper

    F32 = mybir.dt.float32
    BF16 = mybir.dt.bfloat16
    I32 = mybir.dt.int32
    Tanh = mybir.ActivationFunctionType.Tanh

    nc = bacc.Bacc("TRN2", target_bir_lowering=False, debug=False,
                   num_devices=N_CORES)

    # ---- I/O ----
    whT = nc.dram_tensor("whT", [P, KT * MT * P], BF16, kind="ExternalInput")
    wxT = nc.dram_tensor("wxT", [P, KT * MT * P], BF16, kind="ExternalInput")
    bias = nc.dram_tensor("bias", [P, MT], F32, kind="ExternalInput")
    carry = nc.dram_tensor("carry", [ROUNDS, P, KT * BC], mybir.dt.uint8, kind="ExternalInput")
    cinit = nc.dram_tensor("cinit", [ROUNDS, P, KT * BC], BF16, kind="ExternalInput")
    gidx0 = nc.dram_tensor("gidx0", [P, 1], I32, kind="ExternalInput")
    gidx = nc.dram_tensor("gidx", [P, 1], I32, kind="ExternalInput")
    x0t = nc.dram_tensor("x0t", [ROUNDS, P, BLK_COLS], BF16, kind="ExternalInput")
    ag_init = nc.dram_tensor("ag_init", [2 * P, BLK_COLS], BF16, kind="ExternalInput")
    out = nc.dram_tensor("out", [ROUNDS, P, BLK_COLS], F32, kind="ExternalOutput")

    debug = bool(_os.environ.get("RNN_DEBUG"))
    if debug:
        dbg_xb = nc.dram_tensor("dbg_xb", [ROUNDS, P, BLK_COLS], F32,
                                kind="ExternalOutput")
        dbg_xw = nc.dram_tensor("dbg_xw", [ROUNDS, P, BLK_COLS], F32,
                                kind="ExternalOutput")

    ag_ins = [nc.dram_tensor(f"ag_in_{r}", [2 * P, BLK_COLS], BF16)
              for r in range(ROUNDS - 1)]
    ag_outs = [nc.dram_tensor(f"ag_out_{r}", [N_CORES * 2 * P, BLK_COLS], BF16,
                              addr_space="Shared")
               for r in range(ROUNDS - 1)]

    with tile.TileContext(nc) as tc:
        with (
            tc.tile_pool(name="const", bufs=1) as cpool,
            tc.tile_pool(name="xblk", bufs=2) as xpool,
            tc.tile_pool(name="xw", bufs=1) as xwpool,
            tc.tile_pool(name="blk", bufs=1) as blkpool,
            tc.tile_pool(name="hs", bufs=2) as hspool,
            tc.tile_pool(name="o32", bufs=2) as opool,
            tc.tile_pool(name="z", bufs=4) as zpool,
            tc.tile_pool(name="psr", bufs=4, space="PSUM") as prpool,
            tc.tile_pool(name="psp", bufs=2, space="PSUM") as pppool,
        ):
            wh_sb = cpool.tile([P, KT, MT, P], BF16, tag="wh")
            nc.sync.dma_start(wh_sb[:], whT.ap().rearrange("p (k m q) -> p k m q", k=KT, m=MT))
            wx_sb = cpool.tile([P, KT, MT, P], BF16, tag="wx")
            nc.sync.dma_start(wx_sb[:], wxT.ap().rearrange("p (k m q) -> p k m q", k=KT, m=MT))
            bias_sb = cpool.tile([P, MT], F32, tag="bias")
            nc.sync.dma_start(bias_sb[:], bias[:])
            carry_sb = cpool.tile([P, ROUNDS, KT * BC], mybir.dt.uint8, tag="carry")
            nc.sync.dma_start(carry_sb[:], carry.ap().rearrange("r p c -> p r c"))
            cinit_sb = cpool.tile([P, ROUNDS, KT * BC], BF16, tag="cinit")
            nc.sync.dma_start(cinit_sb[:], cinit.ap().rearrange("r p c -> p r c"))
            gidx0_sb = cpool.tile([P, 1], I32, tag="gidx0")
            nc.sync.dma_start(gidx0_sb[:], gidx0[:])
            gidx_sb = cpool.tile([P, 1], I32, tag="gidx")
            nc.sync.dma_start(gidx_sb[:], gidx[:])

            # two persistent block buffers, alternated by round parity
            blkA = blkpool.tile([P, MT, T, BC], BF16, tag="blkA")
            blkB = blkpool.tile([P, MT, T, BC], BF16, tag="blkB")
            nc.vector.memset(blkA[:], 0.0)
            nc.vector.memset(blkB[:], 0.0)

            xw_sb = xwpool.tile([P, MT, T, BC], F32, tag="xw")

            cc_prev = None
            for r in range(ROUNDS):
                cur = blkA if r % 2 == 0 else blkB
                prev = blkB if r % 2 == 0 else blkA

                # ---- 1. gather input block from previous round's AG ----
                src = ag_init if r == 0 else ag_outs[r - 1]
                idx = gidx0_sb if r == 0 else gidx_sb
                xblk = xpool.tile([P, KT * T * BC], BF16, tag="xblk")
                g = nc.gpsimd.indirect_dma_start(
                    out=xblk[:],
                    out_offset=None,
                    in_=src[:],
                    in_offset=bass.IndirectOffsetOnAxis(ap=idx[:, :1], axis=0),
                )
                if cc_prev is not None:
                    add_dep_helper(g.ins, cc_prev.ins, sync=True, reason="gather after AG")

                # ---- 2. projection: xw[m] = sum_k WxT(k,m).T @ xblk[k] + bias[m] ----
                for m in range(MT):
                    pp = pppool.tile([P, T, BC], mybir.dt.float32, tag="pp")
                    for k in range(KT):
                        nc.tensor.matmul(
                            pp[:],
                            wx_sb[:, k, m, :],
                            xblk[:, k * T * BC:(k + 1) * T * BC],
                            start=(k == 0),
                            stop=(k == KT - 1),
                        )
                    nc.vector.tensor_tensor(
                        out=xw_sb[:, m],
                        in0=pp[:],
                        in1=bias_sb[:, m, None].to_broadcast((P, T, BC)),
                        op=mybir.AluOpType.add,
                    )

                if debug:
                    dxb = opool.tile([P, BLK_COLS], F32, tag="dxb")
                    nc.vector.tensor_copy(dxb[:], xblk[:])
                    nc.sync.dma_start(dbg_xb[r], dxb[:])
                    dxw = opool.tile([P, BLK_COLS], F32, tag="dxw")
                    nc.vector.tensor_copy(dxw[:], xw_sb[:])
                    nc.sync.dma_start(dbg_xw[r], dxw[:])

                # ---- 3. h_start = carry ? prev_block_tail : cinit ----
                hstart = hspool.tile([P, KT * BC], BF16, tag="hs")
                nc.vector.tensor_copy(hstart[:], cinit_sb[:, r])
                nc.vector.copy_predicated(
                    hstart[:], carry_sb[:, r], prev[:, :, T - 1, :]
                )

                # ---- 4. recurrence over T steps ----
                for t in range(T):
                    for half in range(2):
                        ps = prpool.tile([P, 4, BC], mybir.dt.float32, tag="ps")
                        # One accumulation group per PSUM bank: start=True only
                        # on the very first matmul (it clears has_written for
                        # the WHOLE bank); later regions overwrite-on-clear
                        # then accumulate. k-outer so the clear runs first.
                        first_mm = None
                        for k in range(KT):
                            if t == 0:
                                rhs = hstart[:, k * BC:(k + 1) * BC]
                            else:
                                rhs = cur[:, k, t - 1, :]
                            for mi in range(4):
                                m = half * 4 + mi
                                mm = nc.tensor.matmul(
                                    ps[:, mi, :],
                                    wh_sb[:, k, m, :],
                                    rhs,
                                    start=(k == 0 and mi == 0),
                                    stop=(k == KT - 1 and mi == 3),
                                    skip_group_check=True,
                                )
                                if first_mm is None:
                                    first_mm = mm
                                elif k == 0:
                                    add_dep_helper(mm.ins, first_mm.ins, sync=False,
                                                   reason="bank clear first")
                        z = zpool.tile([P, 4, BC], mybir.dt.float32, tag="z")
                        nc.vector.tensor_tensor(
                            out=z[:],
                            in0=ps[:],
                            in1=xw_sb[:, half * 4:(half + 1) * 4, t, :],
                            op=mybir.AluOpType.add,
                        )
                        nc.scalar.activation(
                            cur[:, half * 4:(half + 1) * 4, t, :], z[:], Tanh
                        )

                # ---- 5. write fp32 output block ----
                o32 = opool.tile([P, MT * T * BC], F32, tag="o32")
                nc.vector.tensor_copy(o32[:], cur[:])
                nc.sync.dma_start(out[r], o32[:])

                # ---- 6. contribute to AG (block + x-feed) and trigger ----
                if r < ROUNDS - 1:
                    d1 = nc.sync.dma_start(
                        ag_ins[r][0:P, :],
                        cur[:].rearrange("p m t b -> p (m t b)"),
                    )
                    d2 = nc.sync.dma_start(ag_ins[r][P:2 * P, :], x0t[r + 1])
                    cc = nc.gpsimd.collective_compute(
                        "AllGather",
                        mybir.AluOpType.bypass,
                        replica_groups=[list(range(N_CORES))],
                        ins=[ag_ins[r][:]],
                        outs=[ag_outs[r][:]],
                    )
                    add_dep_helper(cc.ins, d1.ins, sync=True, reason="AG after blk dma")
                    add_dep_helper(cc.ins, d2.ins, sync=True, reason="AG after feed dma")
                    cc_prev = cc
    nc.compile()
    return nc


def _prep_inputs(X, h0s, W, b):
    """Build the 8 per-core input maps."""
    in_maps = []
    for c in range(N_CORES):
        s, j = c // 2, c % 2
        Wl = np.asarray(W[s], dtype=np.float32)
        Wx, Wh = Wl[:, :D], Wl[:, D:]

        def tiles(M):  # M: [e, d] -> lhsT tiles [p, (k, m, q)]
            A = M.reshape(MT, P, KT, P)          # [m, q, k, p]
            return np.ascontiguousarray(
                A.transpose(3, 2, 0, 1).reshape(P, KT * MT * P)).astype(BF)

        whT = tiles(Wh)
        wxT = tiles(Wx)
        bias = np.ascontiguousarray(
            np.asarray(b[s], np.float32).reshape(MT, P).T)

        hin = np.asarray(h0s[s, BC * j:BC * (j + 1)], np.float32)  # [b, d]
        hinit = np.ascontiguousarray(
            hin.reshape(BC, KT, P).transpose(2, 1, 0).reshape(P, KT * BC)).astype(BF)

        carry = np.zeros((ROUNDS, P, KT * BC), np.uint8)
        cinit = np.zeros((ROUNDS, P, KT * BC), BF)
        for r in range(ROUNDS):
            if r > s:
                carry[r] = 1
            else:
                cinit[r] = hinit

        x0t = np.zeros((ROUNDS, P, BLK_COLS), BF)
        ag_init = np.zeros((2 * P, BLK_COLS), BF)
        if s == 0:
            Xj = np.asarray(X[BC * j:BC * (j + 1)], np.float32)  # [b, L, d]
            # [b, q, t, k, p] -> [q, p, k, t, b]
            Xb = Xj.reshape(BC, NB, T, KT, P).transpose(1, 4, 3, 2, 0)
            Xb = np.ascontiguousarray(Xb.reshape(NB, P, BLK_COLS)).astype(BF)
            x0t[1:NB] = Xb[1:]
            # block 0 goes into ag_init's feed half
            ag_init[P:2 * P, :] = Xb[0]
            gidx0 = (P + np.arange(P, dtype=np.int32)).reshape(P, 1)
            gidx = (c * 2 * P + P + np.arange(P, dtype=np.int32)).reshape(P, 1)
        else:
            gidx0 = np.arange(P, dtype=np.int32).reshape(P, 1)
            gidx = ((c - 2) * 2 * P + np.arange(P, dtype=np.int32)).reshape(P, 1)

        in_maps.append({
            "whT": whT, "wxT": wxT, "bias": bias,
            "carry": carry, "cinit": cinit,
            "gidx0": gidx0, "gidx": gidx,
            "x0t": x0t, "ag_init": ag_init,
        })
    return in_maps


def _extract(results):
    """Assemble full output [B, L, D] from stage-3 cores (6, 7)."""
    Y = np.empty((B, L, D), np.float32)
    for j in range(2):
        o = results[6 + j]["out"][NL - 1:NL - 1 + NB]   # [q, p, cols]
        o = o.reshape(NB, P, MT, T, BC).transpose(4, 0, 3, 2, 1)  # [b,q,t,m,p]
        Y[BC * j:BC * (j + 1)] = o.reshape(BC, L, D)
    return Y


def kernel(X, h0s, W, b, _trace=False):
    from concourse.bass_utils import run_bass_kernel_spmd

    if "nc" not in _cache:
        _cache["nc"] = _build()
    nc = _cache["nc"]
    in_maps = _prep_inputs(np.asarray(X), np.asarray(h0s), np.asarray(W),
                           np.asarray(b))
    res = run_bass_kernel_spmd(nc, in_maps, core_ids=list(range(N_CORES)),
                               trace=_trace)
    _cache["last_results"] = res
    return _extract(res.results)



# revision 12
# speedup vs baseline: 1.0761x; 1.0761x over previous
"""4-layer tanh RNN on 8 Trainium2 NeuronCores.

Strategy: 4-stage layer pipeline x 2-way batch split (core c = layer c//2,
batch half c%2). Time is processed in T=32-step blocks over ROUNDS =
NB + 2*(NL-1) rounds (lag-2 consumption: a block produced in round r
travels through an AllGather during round r+1 and is consumed in round
r+2, so the collective is fully off the critical path).

Per round, a core runs the 32-step recurrence for its layer reading the
step input projection xw directly from PSUM (pre-accumulated there by the
previous round's interleaved projection matmuls — no vector add on the
step critical path: matmuls accumulate onto xw in PSUM, one tanh per
(step, m-half) reads PSUM and writes the bf16 h tile). The projection of
the NEXT round's block (gathered early via the lag-2 AG) is interleaved
into the second half of the round's steps to fill PE gaps. Bias is folded
in as one extra rank-1 matmul per (m, parity) using a ones-vector rhs.

The AllGather carries one P-row block per core (plus a junk row for
scatter targets): cores 0-5 contribute their output block for the next
stage; cores 6,7 contribute the stage-0 input feed (host-staged X blocks),
which cores 0,1 gather. Per-core *data* (gather/scatter indices,
carry/init masks) encodes each core's role; the instruction stream is
SPMD-uniform.

Compute dtype bf16 (fp8 weights tested: 12% end-to-end error, rejected),
fp32 PSUM accumulation, fp32 tanh, bf16 outputs converted on host.
"""
import sys
import numpy as np

if "/opt/trn_rl_repo" not in sys.path:
    sys.path.insert(0, "/opt/trn_rl_repo")

import ml_dtypes

BF = ml_dtypes.bfloat16

B, L, D, NL = 16, 512, 1024, 4
P = 128
KT = D // P          # 8 contraction tiles
MT = D // P          # 8 output tiles
BC = B // 2          # 8 batch rows per core
T = 32               # timesteps per block
NB = L // T          # 16 blocks
LAG = 2              # rounds from production to consumption
ROUNDS = NB + LAG * (NL - 1)   # 22
N_CORES = 8
BLK_COLS = KT * T * BC         # 2048; col = t*(KT*BC) + k*BC + bc
RP = P + 1                     # AG rows per core (row P = scatter junk)

_cache = {}


def _build():
    import concourse.bass as bass
    import concourse.mybir as mybir
    import concourse.tile as tile
    from concourse import bacc
    from concourse.tile import add_dep_helper

    F32 = mybir.dt.float32
    BF16 = mybir.dt.bfloat16
    I32 = mybir.dt.int32
    Tanh = mybir.ActivationFunctionType.Tanh

    nc = bacc.Bacc("TRN2", target_bir_lowering=False, debug=False,
                   num_devices=N_CORES)

    # ---- I/O ----
    whT = nc.dram_tensor("whT", [P, KT * MT * P], BF16, kind="ExternalInput")
    wxT = nc.dram_tensor("wxT", [P, KT * MT * P], BF16, kind="ExternalInput")
    biasT = nc.dram_tensor("biasT", [P, MT * P], BF16, kind="ExternalInput")
    carry = nc.dram_tensor("carry", [ROUNDS, P, KT * BC], mybir.dt.uint8,
                           kind="ExternalInput")
    cinit = nc.dram_tensor("cinit", [ROUNDS, P, KT * BC], BF16,
                           kind="ExternalInput")
    gidx = nc.dram_tensor("gidx", [P, 1], I32, kind="ExternalInput")
    sidx_blk = nc.dram_tensor("sidx_blk", [P, 1], I32, kind="ExternalInput")
    sidx_feed = nc.dram_tensor("sidx_feed", [P, 1], I32, kind="ExternalInput")
    x0t = nc.dram_tensor("x0t", [ROUNDS, P, BLK_COLS], BF16,
                         kind="ExternalInput")
    ag_init = nc.dram_tensor("ag_init", [LAG, P, BLK_COLS], BF16,
                             kind="ExternalInput")
    out = nc.dram_tensor("out", [ROUNDS, P, BLK_COLS], BF16,
                         kind="ExternalOutput")

    NAG = ROUNDS - LAG  # 20 collectives
    ag_ins = [nc.dram_tensor(f"ag_in_{r}", [RP, BLK_COLS], BF16)
              for r in range(NAG)]
    ag_outs = [nc.dram_tensor(f"ag_out_{r}", [N_CORES * RP, BLK_COLS], BF16,
                              addr_space="Shared")
               for r in range(NAG)]

    with tile.TileContext(nc) as tc:
        with (
            tc.tile_pool(name="const", bufs=1) as cpool,
            tc.tile_pool(name="hs", bufs=2) as hspool,
            tc.tile_pool(name="ps", bufs=1, space="PSUM") as pspool,
        ):
            wh_sb = cpool.tile([P, KT, MT, P], BF16, tag="wh")
            nc.sync.dma_start(wh_sb[:], whT.ap().rearrange(
                "p (k m q) -> p k m q", k=KT, m=MT))
            wx_sb = cpool.tile([P, KT, MT, P], BF16, tag="wx")
            nc.sync.dma_start(wx_sb[:], wxT.ap().rearrange(
                "p (k m q) -> p k m q", k=KT, m=MT))
            bias_sb = cpool.tile([P, MT, P], BF16, tag="bias")
            nc.sync.dma_start(bias_sb[:], biasT.ap().rearrange(
                "p (m q) -> p m q", m=MT))
            carry_sb = cpool.tile([P, ROUNDS, KT * BC], mybir.dt.uint8,
                                  tag="carry")
            nc.sync.dma_start(carry_sb[:], carry.ap().rearrange("r p c -> p r c"))
            cinit_sb = cpool.tile([P, ROUNDS, KT * BC], BF16, tag="cinit")
            nc.sync.dma_start(cinit_sb[:], cinit.ap().rearrange("r p c -> p r c"))
            gidx_sb = cpool.tile([P, 1], I32, tag="gidx")
            nc.sync.dma_start(gidx_sb[:], gidx[:])
            sblk_sb = cpool.tile([P, 1], I32, tag="sblk")
            nc.sync.dma_start(sblk_sb[:], sidx_blk[:])
            sfeed_sb = cpool.tile([P, 1], I32, tag="sfeed")
            nc.sync.dma_start(sfeed_sb[:], sidx_feed[:])
            ones_sb = cpool.tile([P, P], BF16, tag="ones")
            nc.vector.memset(ones_sb[:], 1.0)

            # h output tiles, double-buffered by round parity.
            # layout [P, t, m, bc]
            curA = cpool.tile([P, T, MT, BC], BF16, tag="curA")
            curB = cpool.tile([P, T, MT, BC], BF16, tag="curB")
            nc.vector.memset(curA[:], 0.0)
            nc.vector.memset(curB[:], 0.0)

            # xw PSUM tiles: [P, bank(4), t//2(16), m'(4), bc(8)] fp32.
            # bank = 2*(m//512-half) + t%2; 4 banks per tile, 2 tiles = all 8.
            psA = pspool.tile([P, 4, T // 2, 4, BC], F32, tag="psA")
            psB = pspool.tile([P, 4, T // 2, 4, BC], F32, tag="psB")

            xblks = [cpool.tile([P, T, KT, BC], BF16, tag=f"xblk{i}", name=f"xblk{i}")
                     for i in range(2)]
            feeds = [cpool.tile([P, BLK_COLS], BF16, tag=f"feed{i}", name=f"feed{i}")
                     for i in range(2)]

            # ---- projection emission helper -------------------------------
            def proj_jobs(r):
                """Yield thunks emitting proj/bias MMs for round r's xw.

                Writes ps[r%2]; reads xblk tile of round r. First MM per
                bank has start=True (clears the bank); all others depend on
                it executing first.
                """
                ps = psA if r % 2 == 0 else psB
                xb = xblks[r % 2]
                clear_mm = [None, None, None, None]

                def mk(h, mi, k, par):
                    def emit():
                        b = 2 * h + par
                        m = 4 * h + mi
                        first = (mi == 0 and k == 0)
                        mm = nc.tensor.matmul(
                            ps[:, b, :, mi, :],
                            wx_sb[:, k, m, :],
                            xb[:, par::2, k, :],
                            start=(first and clear_mm[b] is None),
                            stop=False,
                            skip_group_check=True,
                        )
                        if clear_mm[b] is None:
                            clear_mm[b] = mm
                        elif first:
                            add_dep_helper(mm.ins, clear_mm[b].ins, sync=False,
                                           reason="bank clear first")
                        return mm
                    return emit

                def mk_bias(h, mi, par):
                    # rank-1: lhsT = bias tile (only partition 0 nonzero),
                    # rhs = ones => adds b[m*128+o] to every (t'', bc) column
                    def emit():
                        b = 2 * h + par
                        mm = nc.tensor.matmul(
                            ps[:, b, :, mi, :],
                            bias_sb[:, 4 * h + mi, :],
                            ones_sb[:],
                            start=False,
                            stop=False,
                            skip_group_check=True,
                        )
                        add_dep_helper(mm.ins, clear_mm[b].ins, sync=False,
                                       reason="bias after clear")
                        return mm
                    return emit

                jobs = []
                for h in range(2):
                    for par in range(2):
                        for mi in range(4):
                            for k in range(KT):
                                jobs.append(mk(h, mi, k, par))
                # bias rank-1 matmuls last (accumulate onto projected xw)
                for h in range(2):
                    for par in range(2):
                        for mi in range(4):
                            jobs.append(mk_bias(h, mi, par))
                return jobs

            # ---- prologue: gather + project block for round 0 -------------
            nc.sync.dma_start(xblks[0][:], ag_init[0].rearrange(
                "p (t k c) -> p t k c", t=T, k=KT))
            for j in proj_jobs(0):
                j()

            cc_list = []
            for r in range(ROUNDS):
                cur = curA if r % 2 == 0 else curB
                prev = curB if r % 2 == 0 else curA
                ps = psA if r % 2 == 0 else psB

                # ---- early, off critical path ----
                # feed for this round's AG contribution
                if r < NAG:
                    nc.sync.dma_start(feeds[r % 2][:], x0t[r])
                    sc_feed = nc.gpsimd.indirect_dma_start(
                        out=ag_ins[r][:],
                        out_offset=bass.IndirectOffsetOnAxis(
                            ap=sfeed_sb[:, :1], axis=0),
                        in_=feeds[r % 2][:],
                        in_offset=None,
                    )
                # gather next round's input block
                if r + 1 < ROUNDS:
                    nxb = xblks[(r + 1) % 2]
                    if r + 1 < LAG:
                        nc.sync.dma_start(nxb[:], ag_init[r + 1].rearrange(
                            "p (t k c) -> p t k c", t=T, k=KT))
                    else:
                        g = nc.gpsimd.indirect_dma_start(
                            out=nxb[:].rearrange("p t k c -> p (t k c)"),
                            out_offset=None,
                            in_=ag_outs[r - 1][:],
                            in_offset=bass.IndirectOffsetOnAxis(
                                ap=gidx_sb[:, :1], axis=0),
                        )
                        add_dep_helper(g.ins, cc_list[r - 1].ins, sync=True,
                                       reason="gather after AG")

                # ---- h_start = carry ? prev block tail : cinit ----
                hstart = hspool.tile([P, KT * BC], BF16, tag="hs")
                nc.vector.tensor_copy(hstart[:], cinit_sb[:, r])
                nc.vector.copy_predicated(
                    hstart[:], carry_sb[:, r],
                    prev[:, T - 1, :, :].rearrange("p m c -> p (m c)"))

                # ---- 32 recurrence steps; proj(r+1) interleaved ----
                jobs = proj_jobs(r + 1) if r + 1 < ROUNDS else []
                ji = 0
                # interleave proj MMs into the last 16 steps' 32 half-slots
                # (not earlier: the gather they depend on rides on AG(r-1),
                # and a waiting MM head-of-line-blocks the PE FIFO)
                T_ILV = 16
                nslots = (T - T_ILV) * 2
                per_slot = (len(jobs) + nslots - 1) // nslots if jobs else 0
                for t in range(T):
                    par = t % 2
                    for h in range(2):
                        bnk = 2 * h + par
                        reg = ps[:, bnk, t // 2, :, :]
                        for k in range(KT):
                            if t == 0:
                                rhs = hstart[:, k * BC:(k + 1) * BC]
                            else:
                                rhs = cur[:, t - 1, k, :]
                            for mi in range(4):
                                nc.tensor.matmul(
                                    reg[:, mi, :],
                                    wh_sb[:, k, 4 * h + mi, :],
                                    rhs,
                                    start=False,
                                    stop=False,
                                    skip_group_check=True,
                                )
                        nc.scalar.activation(
                            cur[:, t, 4 * h:4 * h + 4, :], reg, Tanh)
                        if t >= T_ILV:
                            for _ in range(per_slot):
                                if ji < len(jobs):
                                    jobs[ji]()
                                    ji += 1
                while ji < len(jobs):
                    jobs[ji]()
                    ji += 1

                # ---- write output block (bf16; host converts) ----
                nc.sync.dma_start(
                    out[r], cur[:].rearrange("p t m c -> p (t m c)"))

                # ---- contribute block + trigger AG ----
                if r < NAG:
                    sc_blk = nc.gpsimd.indirect_dma_start(
                        out=ag_ins[r][:],
                        out_offset=bass.IndirectOffsetOnAxis(
                            ap=sblk_sb[:, :1], axis=0),
                        in_=cur[:].rearrange("p t m c -> p (t m c)"),
                        in_offset=None,
                    )
                    cc = nc.gpsimd.collective_compute(
                        "AllGather",
                        mybir.AluOpType.bypass,
                        replica_groups=[list(range(N_CORES))],
                        ins=[ag_ins[r][:]],
                        outs=[ag_outs[r][:]],
                    )
                    add_dep_helper(cc.ins, sc_blk.ins, sync=True,
                                   reason="AG after block scatter")
                    add_dep_helper(cc.ins, sc_feed.ins, sync=True,
                                   reason="AG after feed scatter")
                    cc_list.append(cc)
    nc.compile()
    return nc


def _prep_inputs(X, h0s, W, b):
    """Build the 8 per-core input maps."""
    in_maps = []

    def xb_layout(Xj):
        # [bc, L, d] -> per block [P, (t, k, bc)]
        A = Xj.reshape(BC, NB, T, KT, P)          # [bc, nb, t, k, p]
        A = A.transpose(1, 4, 2, 3, 0)            # [nb, p, t, k, bc]
        return np.ascontiguousarray(A.reshape(NB, P, BLK_COLS)).astype(BF)

    for c in range(N_CORES):
        s, j = c // 2, c % 2
        Wl = np.asarray(W[s], dtype=np.float32)
        Wx, Wh = Wl[:, :D], Wl[:, D:]

        def tiles(M):  # M: [e, d] -> lhsT tiles [p, (k, m, q)]
            A = M.reshape(MT, P, KT, P)           # [m, q, k, p]
            return np.ascontiguousarray(
                A.transpose(3, 2, 0, 1).reshape(P, KT * MT * P)).astype(BF)

        whT = tiles(Wh)
        wxT = tiles(Wx)
        biasT = np.zeros((P, MT, P), np.float32)
        biasT[0] = np.asarray(b[s], np.float32).reshape(MT, P)
        biasT = biasT.reshape(P, MT * P).astype(BF)

        hin = np.asarray(h0s[s, BC * j:BC * (j + 1)], np.float32)  # [bc, d]
        hinit = np.ascontiguousarray(
            hin.reshape(BC, KT, P).transpose(2, 1, 0).reshape(P, KT * BC)
        ).astype(BF)

        carry = np.zeros((ROUNDS, P, KT * BC), np.uint8)
        cinit = np.zeros((ROUNDS, P, KT * BC), BF)
        for r in range(ROUNDS):
            if r > LAG * s:
                carry[r] = 1
            elif r == LAG * s:
                cinit[r] = hinit

        x0t = np.zeros((ROUNDS, P, BLK_COLS), BF)
        ag_init = np.zeros((LAG, P, BLK_COLS), BF)
        if s == 0:
            Xb = xb_layout(np.asarray(X[BC * j:BC * (j + 1)], np.float32))
            ag_init[0] = Xb[0]
            ag_init[1] = Xb[1]
        if s == 3:
            # cores 6,7 carry the stage-0 feed for half j: block r+2 at round r
            Xb = xb_layout(np.asarray(X[BC * j:BC * (j + 1)], np.float32))
            for r in range(ROUNDS):
                if r + 2 < NB:
                    x0t[r] = Xb[r + 2]

        if s == 0:
            gidx = ((6 + c) * RP + np.arange(P, dtype=np.int32)).reshape(P, 1)
        else:
            gidx = ((c - 2) * RP + np.arange(P, dtype=np.int32)).reshape(P, 1)
        if s == 3:
            sidx_blk = np.full((P, 1), P, np.int32)      # junk row
            sidx_feed = np.arange(P, dtype=np.int32).reshape(P, 1)
        else:
            sidx_blk = np.arange(P, dtype=np.int32).reshape(P, 1)
            sidx_feed = np.full((P, 1), P, np.int32)     # junk row

        in_maps.append({
            "whT": whT, "wxT": wxT, "biasT": biasT,
            "carry": carry, "cinit": cinit,
            "gidx": gidx, "sidx_blk": sidx_blk, "sidx_feed": sidx_feed,
            "x0t": x0t, "ag_init": ag_init,
        })
    return in_maps


def _extract(results):
    """Assemble full output [B, L, D] from stage-3 cores (6, 7)."""
    Y = np.empty((B, L, D), np.float32)
    r0 = LAG * (NL - 1)
    for j in range(2):
        o = results[6 + j]["out"][r0:r0 + NB]            # [nb, p, (t m c)]
        o = o.reshape(NB, P, T, MT, BC).astype(np.float32)
        o = o.transpose(4, 0, 2, 3, 1)                   # [bc, nb, t, m, p]
        Y[BC * j:BC * (j + 1)] = o.reshape(BC, L, D)
    return Y


def kernel(X, h0s, W, b, _trace=False):
    from concourse.bass_utils import run_bass_kernel_spmd

    if "nc" not in _cache:
        _cache["nc"] = _build()
    nc = _cache["nc"]
    in_maps = _prep_inputs(np.asarray(X), np.asarray(h0s), np.asarray(W),
                           np.asarray(b))
    res = run_bass_kernel_spmd(nc, in_maps, core_ids=list(range(N_CORES)),
                               trace=_trace)
    _cache["last_results"] = res
    return _extract(res.results)
